# revision 8
# baseline (speedup 1.0000x reference)
"""ContextRetentionLayer Trainium2 kernel — fp8 DoubleRow version.

Reference computation (per token t, d=1024, W=512 memory slots):
    s[t, w]   = (x[t] . mb[w]) / 32
    attn[t]   = softmax_w(s[t])
    r[t]      = sum_w attn[t, w] * mb[w]
    g[t]      = sigmoid(x[t] @ gw.T + gb)
    out[t]    = g[t] * x[t] + (1 - g[t]) * r[t]

Sharding: 4x4096 = 16384 tokens split evenly across 8 cores (2048 each);
memory_bank / gate weights replicated.

All three big matmuls (scores, retrieved, gate) run in fp8 e4m3 with
perf_mode=DoubleRow: operands live as [128, KC, free] with K-chunk pairs
contracted 256 at a time at ~2x the bf16/f32r column rate.  Tolerance is
2e-2; CPU sim of this exact quantization measures rel err ~1.8e-2
(gate-path dominated; see sim_fp8.py).

Numerics:
  x8 = e4m3(x); mb8 = e4m3(mb); gw8 = e4m3(32*gw)
  s_ps[w,t] = sum_d mb8T . x8          (PSUM f32, = 32*s approx)
  at8       = e4m3(exp(s_ps/32))
  den_ps    = sum_w at8/64             (ones lhsT = 1/64)
  rb        = bcast(1/den_ps) = 64/den
  at8      *= rb                       (in place -> 64*attn, e4m3; keeps
                                        attn out of e4m3 subnormals)
  r_ps      = sum_w mb8 . at8          (= 64*r)
  z_ps      = sum_d gw8T . x8          (= 32*z approx)
  g         = sigmoid(z_ps/32 + gb)
  out       = bf16( r_ps/64 + g*(xc - r_ps/64) ),  xc = bf16(x)
Combine is 3 fused DVE ops (scalar_tensor_tensor folds the 1/64).

Softmax max-subtraction is skipped: scores/32 ~ N(0,1), exp <= ~110 < 240
(e4m3 max), no overflow.  Weight-stationary loop order (weights reused
across 2-4 token tiles per LDWEIGHTS) since DoubleRow disables FWL.
"""

import numpy as np
import ml_dtypes

import concourse.bass as bass
import concourse.tile as tile
from concourse import bacc, bass_utils, mybir
from concourse.bass import ts

AF = mybir.ActivationFunctionType
ALU = mybir.AluOpType
F32 = mybir.dt.float32
BF16 = mybir.dt.bfloat16
F8E4 = mybir.dt.float8e4
E4NP = ml_dtypes.float8_e4m3
BFNP = ml_dtypes.bfloat16

N_CORES = 8
B, S, D = 4, 4096, 1024
W = 512
T_CORE = (B * S) // N_CORES  # 2048 tokens per core
T_TILE = 512                 # moving free dim per matmul (DR pairs: rhs 1024)
NT = T_CORE // T_TILE        # 4 token tiles
DC = D // 128                # 8 chunks of the embed dim
WC = W // 128                # 4 chunks of the memory window

# "dr": natural [128, KC, M] weights, perf_mode=DoubleRow
# "dri": host-interleaved weights, perf_mode=DoubleRowSwInterleave
PERF = "dri"
GATE_BF16 = False            # fallback: gate matmul in bf16 (better precision)
PROBE = "full"               # "full" | "mm" (matmuls+exp only) | "mmnorm"

_PM = {
    "dr": mybir.MatmulPerfMode.DoubleRow,
    "dri": mybir.MatmulPerfMode.DoubleRowSwInterleave,
}


def _body(tc: tile.TileContext, reps: int = 1):
    nc = tc.nc

    x8 = nc.dram_tensor("x8", (D, T_CORE), F8E4, kind="ExternalInput").ap()
    xc = nc.dram_tensor("xc", (D, T_CORE), BF16, kind="ExternalInput").ap()
    gb = nc.dram_tensor("gb", (D,), F32, kind="ExternalInput").ap()
    ones = nc.dram_tensor("ones", (128, 1), F8E4, kind="ExternalInput").ap()
    if PERF == "dr":
        mbt = nc.dram_tensor("mbt", (D, W), F8E4, kind="ExternalInput").ap()
        mbw = nc.dram_tensor("mbw", (W, D), F8E4, kind="ExternalInput").ap()
    else:
        mbt = nc.dram_tensor("mbt", (128, DC // 2, WC, 256), F8E4,
                             kind="ExternalInput").ap()
        mbw = nc.dram_tensor("mbw", (128, WC // 2, DC, 256), F8E4,
                             kind="ExternalInput").ap()
    if GATE_BF16:
        gwt = nc.dram_tensor("gwt", (D, D), BF16, kind="ExternalInput").ap()
    elif PERF == "dr":
        gwt = nc.dram_tensor("gwt", (D, D), F8E4, kind="ExternalInput").ap()
    else:
        gwt = nc.dram_tensor("gwt", (128, DC // 2, DC, 256), F8E4,
                             kind="ExternalInput").ap()
    outT = nc.dram_tensor("outt", (D, T_CORE), BF16, kind="ExternalOutput").ap()

    for _rep in range(reps):
        _emit_once(tc, x8, xc, gb, ones, mbt, mbw, gwt, outT)


def _emit_once(tc, x8, xc, gb, ones, mbt, mbw, gwt, outT):
    nc = tc.nc
    pm = _PM[PERF]
    with (
        tc.tile_pool(name="const", bufs=1) as const,
        tc.tile_pool(name="big", bufs=1) as big,
        tc.tile_pool(name="work", bufs=3) as work,
        tc.tile_pool(name="mm_ps", bufs=7, space="PSUM") as mm_ps,
        tc.tile_pool(name="den_psp", bufs=1, space="PSUM") as den_psp,
    ):
        # ---- SBUF tiles
        if PERF == "dr":
            mbt_s = const.tile([128, DC, W], F8E4)
            mbw_s = const.tile([128, WC, D], F8E4)
        else:
            mbt_s = const.tile([128, DC // 2, WC, 256], F8E4)
            mbw_s = const.tile([128, WC // 2, DC, 256], F8E4)
        if GATE_BF16:
            gwt_s = const.tile([128, DC, D], BF16)
        elif PERF == "dr":
            gwt_s = const.tile([128, DC, D], F8E4)
        else:
            gwt_s = const.tile([128, DC // 2, DC, 256], F8E4)
        gb_s = const.tile([128, DC], F32)
        ones_s = const.tile([128, 1], F8E4)

        x8_s = big.tile([128, DC, T_CORE], F8E4)
        xc_s = big.tile([128, DC, T_CORE], BF16)
        at8_s = big.tile([128, WC, T_CORE], F8E4)
        rd_s = big.tile([1, T_CORE], F32)
        rb_s = big.tile([128, T_CORE], F32)

        x8v = x8.rearrange("(c p) t -> p c t", p=128)
        xcv = xc.rearrange("(c p) t -> p c t", p=128)
        outv = outT.rearrange("(c p) t -> p c t", p=128)

        # ---- need-ordered loads: scores weights + x tile 0 first.
        if PERF == "dr":
            mbtv = mbt.rearrange("(c p) w -> p c w", p=128)
            for c in range(DC):
                nc.sync.dma_start(out=mbt_s[:, c, :], in_=mbtv[:, c, :])
        else:
            nc.sync.dma_start(out=mbt_s, in_=mbt)
        nc.sync.dma_start(out=x8_s[:, :, ts(0, T_TILE)], in_=x8v[:, :, ts(0, T_TILE)])
        nc.sync.dma_start(out=ones_s, in_=ones)
        nc.sync.dma_start(out=gb_s, in_=gb.rearrange("(c p) -> p c", p=128))
        for ti in range(1, NT):
            nc.sync.dma_start(
                out=x8_s[:, :, ts(ti, T_TILE)], in_=x8v[:, :, ts(ti, T_TILE)]
            )
        if PERF == "dr":
            mbwv = mbw.rearrange("(c p) d -> p c d", p=128)
            for c in range(WC):
                nc.sync.dma_start(out=mbw_s[:, c, :], in_=mbwv[:, c, :])
        else:
            nc.sync.dma_start(out=mbw_s, in_=mbw)
        if GATE_BF16 or PERF == "dr":
            gwtv = gwt.rearrange("(c p) e -> p c e", p=128)
            for c in range(DC):
                nc.sync.dma_start(out=gwt_s[:, c, :], in_=gwtv[:, c, :])
        else:
            nc.sync.dma_start(out=gwt_s, in_=gwt)
        for ti in range(NT):
            nc.sync.dma_start(
                out=xc_s[:, :, ts(ti, T_TILE)], in_=xcv[:, :, ts(ti, T_TILE)]
            )

        def score_w(dcp, wc):
            if PERF == "dr":
                return mbt_s[:, 2 * dcp : 2 * dcp + 2, ts(wc, 128)]
            return mbt_s[:, dcp, wc, :]

        def retr_w(wcp, dc):
            if PERF == "dr":
                return mbw_s[:, 2 * wcp : 2 * wcp + 2, ts(dc, 128)]
            return mbw_s[:, wcp, dc, :]

        def gate_w(dcp, ec):
            if PERF == "dr":
                return gwt_s[:, 2 * dcp : 2 * dcp + 2, ts(ec, 128)]
            return gwt_s[:, dcp, ec, :]

        # ---- pass 1: scores + exp.  Weight-stationary: each (wc, dcp)
        # weight pair streams all 4 token tiles before switching.
        for wc in range(WC):
            s_ps = [mm_ps.tile([128, T_TILE], F32, tag="mm", name=f"sps{wc}_{i}")
                    for i in range(NT)]
            for dcp in range(DC // 2):
                for ti in range(NT):
                    nc.tensor.matmul(
                        s_ps[ti],
                        lhsT=score_w(dcp, wc),
                        rhs=x8_s[:, 2 * dcp : 2 * dcp + 2, ts(ti, T_TILE)],
                        start=(dcp == 0),
                        stop=(dcp == DC // 2 - 1),
                        perf_mode=pm,
                    )
            for ti in range(NT):
                nc.scalar.activation(
                    out=at8_s[:, wc, ts(ti, T_TILE)], in_=s_ps[ti],
                    func=AF.Exp, scale=1.0 / 32.0,
                )

        # ---- denominators + normalize (at8 <- 64*attn, in place)
        for ti in range(NT):
            tsl = ts(ti, T_TILE)
            den_ps = den_psp.tile([1, T_TILE], F32, tag="den")
            for wc in range(WC):
                nc.tensor.matmul(
                    den_ps,
                    lhsT=ones_s,
                    rhs=at8_s[:, wc, tsl],
                    start=(wc == 0),
                    stop=(wc == WC - 1),
                )
            rscr = work.tile([1, T_TILE], F32, tag="rscr")
            nc.vector.reciprocal_approx_accurate(
                out=rd_s[:, tsl], in_=den_ps, scratch=rscr
            )
            if PROBE == "mm":
                continue
            nc.gpsimd.partition_broadcast(rb_s[:, tsl], rd_s[:, tsl])
            for wc in range(WC):
                nc.vector.tensor_mul(at8_s[:, wc, tsl], at8_s[:, wc, tsl], rb_s[:, tsl])

        # ---- pass 2: gate + retrieved + combine.  Gate groups (which only
        # need x8) run first so the attn normalize hides behind them; then
        # retrieved+combine groups interleave between remaining gate groups
        # so the DVE combine drains under PE streams.  Weights reused x4.
        g_s = big.tile([128, DC, T_CORE], F32)

        def gate_group(dc):
            z_ps = [mm_ps.tile([128, T_TILE], F32, tag="mm", name=f"zps{dc}_{i}")
                    for i in range(NT)]
            if GATE_BF16:
                for kc in range(DC):
                    for ti in range(NT):
                        nc.tensor.matmul(
                            z_ps[ti],
                            lhsT=gwt_s[:, kc, ts(dc, 128)],
                            rhs=xc_s[:, kc, ts(ti, T_TILE)],
                            start=(kc == 0),
                            stop=(kc == DC - 1),
                        )
            else:
                for dcp in range(DC // 2):
                    for ti in range(NT):
                        nc.tensor.matmul(
                            z_ps[ti],
                            lhsT=gate_w(dcp, dc),
                            rhs=x8_s[:, 2 * dcp : 2 * dcp + 2, ts(ti, T_TILE)],
                            start=(dcp == 0),
                            stop=(dcp == DC // 2 - 1),
                            perf_mode=pm,
                        )
            for ti in range(NT):
                nc.scalar.activation(
                    out=g_s[:, dc, ts(ti, T_TILE)], in_=z_ps[ti], func=AF.Sigmoid,
                    scale=(1.0 if GATE_BF16 else 1.0 / 32.0),
                    bias=gb_s[:, dc : dc + 1],
                )

        def retr_group(dc):
            r_ps = [mm_ps.tile([128, T_TILE], F32, tag="mm", name=f"rps{dc}_{i}")
                    for i in range(NT)]
            for wcp in range(WC // 2):
                for ti in range(NT):
                    nc.tensor.matmul(
                        r_ps[ti],
                        lhsT=retr_w(wcp, dc),
                        rhs=at8_s[:, 2 * wcp : 2 * wcp + 2, ts(ti, T_TILE)],
                        start=(wcp == 0),
                        stop=(wcp == WC // 2 - 1),
                        perf_mode=pm,
                    )
            if PROBE in ("mm", "mmnorm"):
                return
            for ti in range(NT):
                tsl = ts(ti, T_TILE)
                # out = r/64 + g*(xc - r/64), via:
                #   t = r/64 - xc ; u = g*t ; o = r/64 - u
                t = work.tile([128, T_TILE], F32, tag="t")
                nc.vector.scalar_tensor_tensor(
                    out=t, in0=r_ps[ti], scalar=1.0 / 64.0,
                    in1=xc_s[:, dc, tsl],
                    op0=ALU.mult, op1=ALU.subtract,
                )
                u = work.tile([128, T_TILE], F32, tag="u")
                nc.vector.tensor_mul(u, t, g_s[:, dc, tsl])
                o = work.tile([128, T_TILE], BF16, tag="o")
                nc.vector.scalar_tensor_tensor(
                    out=o, in0=r_ps[ti], scalar=1.0 / 64.0, in1=u,
                    op0=ALU.mult, op1=ALU.subtract,
                )
                nc.sync.dma_start(out=outv[:, dc, tsl], in_=o)

        sched = ["G0", "G1", "G2", "R0", "G3", "R1", "G4", "R2", "G5",
                 "R3", "G6", "R4", "G7", "R5", "R6", "R7"]
        for step in sched:
            if step[0] == "G":
                gate_group(int(step[1]))
            else:
                retr_group(int(step[1]))


_NC_CACHE = None


def _build_nc(reps: int = 1):
    global _NC_CACHE
    if reps == 1 and _NC_CACHE is not None:
        return _NC_CACHE
    nc = bacc.Bacc("TRN2", target_bir_lowering=False, debug=False,
                   enable_asserts=False)
    with tile.TileContext(nc) as tc:
        _body(tc, reps)
    nc.compile()
    if reps == 1:
        _NC_CACHE = nc
    return nc


def _interleave(w0, w1):
    """SwInterleave weight layout: out[p, 2c+i] = w_i[p, 127-c]."""
    out = np.empty((w0.shape[0], 256), dtype=w0.dtype)
    out[:, 0::2] = w0[:, ::-1]
    out[:, 1::2] = w1[:, ::-1]
    return out


def make_in_maps(x, memory_bank, gate_w, gate_b):
    x = np.ascontiguousarray(np.asarray(x, np.float32)).reshape(B * S, D)
    mb8 = np.asarray(memory_bank, np.float32).astype(E4NP)       # [W, D]
    gw8 = (np.asarray(gate_w, np.float32) * 32.0).astype(E4NP)   # [E, D]
    gb_n = np.ascontiguousarray(np.asarray(gate_b, np.float32))
    ones_n = np.full((128, 1), 1.0 / 64.0, E4NP)

    if PERF == "dr":
        mbt_n = np.ascontiguousarray(mb8.T)                       # [D, W]
        mbw_n = np.ascontiguousarray(mb8)                         # [W, D]
    else:
        # mbt[p, dcp, wc, 2c+i] = mb8[wc*128 + 127-c, (2dcp+i)*128 + p]
        mbt_n = np.empty((128, DC // 2, WC, 256), E4NP)
        mbw_n = np.empty((128, WC // 2, DC, 256), E4NP)
        for dcp in range(DC // 2):
            for wc in range(WC):
                w0 = mb8[wc * 128 : wc * 128 + 128, (2 * dcp) * 128 : (2 * dcp + 1) * 128].T
                w1 = mb8[wc * 128 : wc * 128 + 128, (2 * dcp + 1) * 128 : (2 * dcp + 2) * 128].T
                mbt_n[:, dcp, wc, :] = _interleave(w0, w1)
        # mbw[p, wcp, dc, 2c+i] = mb8[(2wcp+i)*128 + p, dc*128 + 127-c]
        for wcp in range(WC // 2):
            for dc in range(DC):
                w0 = mb8[(2 * wcp) * 128 : (2 * wcp + 1) * 128, dc * 128 : dc * 128 + 128]
                w1 = mb8[(2 * wcp + 1) * 128 : (2 * wcp + 2) * 128, dc * 128 : dc * 128 + 128]
                mbw_n[:, wcp, dc, :] = _interleave(w0, w1)

    if GATE_BF16:
        gwt_n = np.ascontiguousarray(np.asarray(gate_w, np.float32).T).astype(BFNP)
    elif PERF == "dr":
        gwt_n = np.ascontiguousarray(gw8.T)                       # [D, E]
    else:
        # gwt[p, dcp, ec, 2c+i] = gw8[ec*128 + 127-c, (2dcp+i)*128 + p]
        gwt_n = np.empty((128, DC // 2, DC, 256), E4NP)
        for dcp in range(DC // 2):
            for ec in range(DC):
                w0 = gw8[ec * 128 : ec * 128 + 128, (2 * dcp) * 128 : (2 * dcp + 1) * 128].T
                w1 = gw8[ec * 128 : ec * 128 + 128, (2 * dcp + 1) * 128 : (2 * dcp + 2) * 128].T
                gwt_n[:, dcp, ec, :] = _interleave(w0, w1)

    in_maps = []
    for c in range(N_CORES):
        xs = x[c * T_CORE : (c + 1) * T_CORE]                     # [T, D]
        xsT = np.ascontiguousarray(xs.T)                          # [D, T]
        in_maps.append(
            {
                "x8": xsT.astype(E4NP),
                "xc": xsT.astype(BFNP),
                "gb": gb_n,
                "ones": ones_n,
                "mbt": mbt_n,
                "mbw": mbw_n,
                "gwt": gwt_n,
            }
        )
    return in_maps


def assemble_out(results):
    shards = [results[c]["outt"].astype(np.float32).T for c in range(N_CORES)]
    return np.concatenate(shards, axis=0).reshape(B, S, D)


def kernel(x, memory_bank, gate_w, gate_b, _run_kwargs=None):
    nc = _build_nc()
    in_maps = make_in_maps(x, memory_bank, gate_w, gate_b)
    res = bass_utils.run_bass_kernel_spmd(
        nc, in_maps, core_ids=list(range(N_CORES)), **(_run_kwargs or {})
    )
    out = assemble_out(res.results)
    if _run_kwargs:
        kernel.last_result = res
    return out
